# revision 40
# baseline (speedup 1.0000x reference)
"""BitLinear158 (LayerNorm -> int8 fake-quant -> ternary matmul -> LayerNorm)
on 8 Trainium2 NeuronCores, data-parallel over tokens.

Math notes (vs the fp32 reference):
  - Input LayerNorm's rstd cancels inside the activation quantizer:
        q = round(xn / (max|xn|/127)) = round((x-mu) * 127 / max|x-mu|)
    so the input-side sqrt/reciprocal of the variance is never needed.
  - q in [-127,127] is exact in bf16 and the PE accumulates in fp32.
  - weight_scale is folded into the bf16 weights host-side
    (w = ternary * scale, rounded to bf16).  The final LayerNorm is
    invariant to per-token scales, and the bf16 rounding of the scaled
    ternary weights contributes ~1e-3 relative error -- well inside the
    2e-2 gate.
  - Output is stored as bf16 (LN output is O(1); bf16 adds ~2e-3 rel)
    and upcast to fp32 on the host.
  - round-half-to-even via the fp32 magic-number trick:
    t = fma(v, c, 1.5*2^23); q = t - 1.5*2^23.

Schedule notes:
  - Weights stream in 4 x 2MiB per-chunk DMAs (separate tiles, separate
    completion semaphores) on the gpsimd sw-DGE queue, so early matmuls
    only wait for the chunks they read instead of the whole 8 MiB.
    Fewer chunks also hold fewer DMA-semaphore IDs for the kernel's
    lifetime, easing the rotating semaphore pool the x/store/transpose
    DMAs recycle through (8-chunk and 2-chunk variants both measure
    slower).
  - Queue separation: x loads + output stores ride the scalar HW-DGE
    queue; the sync HW-DGE queue carries ONLY the q transposes.  With x
    loads on sync the transposes queued behind them and the first matmul
    slipped ~10us.
  - Block 0 is special-cased (quant0): half-column x loads on two queues,
    an all-vector LN/quant chain, and PE transposes (via identity matmul
    into a psum scratch slot) instead of the xbar DMA transpose, shaving
    ~10us off the pipeline ramp.  A burst of dummy matmuls warms the PE
    pstate clock during the ramp.
  - quant(b+1) is emitted before the psum-stats of block b so the vector
    engine is not head-of-line blocked on a psum semaphore; the last
    block runs nt-outer matmuls with per-bank stats and a split
    apply/store to shorten the drain.
"""

from contextlib import ExitStack

import numpy as np
import ml_dtypes

N_CORES = 8
B, S, DIN, DOUT = 4, 4096, 2048, 2048
M_TOTAL = B * S
M_PER_CORE = M_TOTAL // N_CORES
P = 128
NBLK = M_PER_CORE // P          # token blocks per core
KT = DIN // P                   # contraction subtiles
NT = DOUT // 512                # psum bank tiles
WCHUNK = 4                      # kt per weight-stream chunk
EPS = 1e-5
MAGIC = float(np.float32(1.5 * 2 ** 23))

_CACHE = {}


def _build_nc(m_per_core=M_PER_CORE):
    key = ("nc", m_per_core)
    if key in _CACHE:
        return _CACHE[key]
    NBLK = m_per_core // P

    import concourse.bacc as bacc
    import concourse.tile as tile
    from concourse import mybir

    f32 = mybir.dt.float32
    bf16 = mybir.dt.bfloat16
    X = mybir.AxisListType.X
    Identity = mybir.ActivationFunctionType.Identity
    Sqrt = mybir.ActivationFunctionType.Sqrt
    Alu = mybir.AluOpType

    nc = bacc.Bacc("TRN2", target_bir_lowering=False, num_devices=N_CORES,
                   name="bitlinear158")
    xs = nc.dram_tensor("xs", [m_per_core, DIN], f32, kind="ExternalInput")
    wt = nc.dram_tensor("wt", [DIN, DOUT], bf16, kind="ExternalInput")
    out = nc.dram_tensor("out", [m_per_core, DOUT], bf16,
                         kind="ExternalOutput")

    with tile.TileContext(nc) as tc, ExitStack() as ctx:
        singles = ctx.enter_context(tc.tile_pool(name="singles", bufs=1))
        xp = ctx.enter_context(tc.tile_pool(name="xp", bufs=2))
        xcp = ctx.enter_context(tc.tile_pool(name="xcp", bufs=3))
        qp = ctx.enter_context(tc.tile_pool(name="qp", bufs=3))
        qtp = ctx.enter_context(tc.tile_pool(name="qtp", bufs=3))
        op_ = ctx.enter_context(tc.tile_pool(name="op", bufs=4))
        stp = ctx.enter_context(tc.tile_pool(name="stp", bufs=24))
        psp = ctx.enter_context(tc.tile_pool(name="psp", bufs=2, space="PSUM"))

        # One tile per weight chunk: each gets its own completion semaphore,
        # so matmuls on early kt don't wait for the whole 8 MiB load.
        wtr = wt.rearrange("(kt p) n -> p kt n", p=P)
        w_sbs = []
        for c in range(KT // WCHUNK):
            ks = slice(c * WCHUNK, (c + 1) * WCHUNK)
            w_c = singles.tile([P, WCHUNK, DOUT], bf16,
                               name=f"w_sb{c}", tag=f"w{c}")
            nc.gpsimd.dma_start(out=w_c, in_=wtr[:, ks, :])
            w_sbs.append(w_c)

        def w_rhs(kt, ncols):
            return w_sbs[kt // WCHUNK][:, kt % WCHUNK, ncols]

        from concourse import masks
        ident = singles.tile([P, P], bf16)
        masks.make_identity(nc, ident)
        eps_t = singles.tile([P, 1], f32)
        nc.vector.memset(eps_t, EPS)
        magic_t = singles.tile([P, 1], f32)
        nc.vector.memset(magic_t, MAGIC)

        # PE warm-up: dummy matmuls against weight chunk 0 while the first
        # block's LN/quant chain runs, so the PE pstate clock is at full
        # speed when the real matmuls arrive.  Results are discarded.
        zlhs = singles.tile([P, P], bf16)
        nc.vector.memset(zlhs, 0.0)
        ps_warm = psp.tile([P, DOUT], f32, name="ps", tag="ps")
        with tc.high_priority():
            for r in range(12):
                nc.tensor.matmul(ps_warm[:, 0:512], lhsT=zlhs,
                                 rhs=w_sbs[0][:, 0, 0:512],
                                 start=True, stop=True)

        def quant0():
            """Block 0, column-split in halves: both half-loads go out on
            separate DMA queues at t=0 and every op is half-width, so the
            critical chain to the first matmul is roughly halved."""
            H = DIN // 2
            x_t = xp.tile([P, DIN], f32)
            nc.sync.dma_start(out=x_t[:, :H], in_=xs[0:P, :H])
            nc.scalar.dma_start(out=x_t[:, H:], in_=xs[0:P, H:])

            sl = stp.tile([P, 1], f32)
            nc.vector.reduce_sum(out=sl, in_=x_t[:, :H], axis=X)
            sr = stp.tile([P, 1], f32)
            nc.vector.reduce_sum(out=sr, in_=x_t[:, H:], axis=X)
            negmu = stp.tile([P, 1], f32)
            nc.vector.tensor_add(negmu, sl, sr)
            nc.vector.tensor_scalar_mul(negmu, negmu, -1.0 / DIN)

            xc_t = xcp.tile([P, DIN], f32)
            amax = stp.tile([P, 1], f32)
            am_r = stp.tile([P, 1], f32)
            nc.vector.tensor_scalar(xc_t[:, :H], x_t[:, :H], negmu, None,
                                    op0=Alu.add)
            nc.vector.tensor_reduce(out=amax, in_=xc_t[:, :H], axis=X,
                                    op=Alu.max, apply_absolute_value=True)
            nc.vector.tensor_scalar(xc_t[:, H:], x_t[:, H:], negmu, None,
                                    op0=Alu.add)
            nc.vector.tensor_reduce(out=am_r, in_=xc_t[:, H:], axis=X,
                                    op=Alu.max, apply_absolute_value=True)
            nc.vector.tensor_max(amax, amax, am_r)

            c127 = stp.tile([P, 1], f32)
            nc.vector.reciprocal(out=c127, in_=amax)
            nc.vector.tensor_scalar_mul(c127, c127, 127.0)

            q_t = qp.tile([P, DIN], bf16)
            qT3 = qtp.tile([P, KT, P], bf16)
            for s in range(2):
                cols = slice(s * H, (s + 1) * H)
                nc.vector.tensor_scalar(xc_t[:, cols], xc_t[:, cols], c127,
                                        MAGIC, op0=Alu.mult, op1=Alu.add)
                nc.vector.tensor_scalar(q_t[:, cols], xc_t[:, cols], MAGIC,
                                        None, op0=Alu.subtract)
            # PE transposes (psum scratch) instead of DMA transposes: the
            # xbar path fires ~10us after its deps are met; the PE path
            # feeds the first matmuls as each 128x128 chunk lands.
            psT = psp.tile([P, KT, P], bf16, name="ps", tag="ps")
            for kt in range(KT):
                nc.tensor.transpose(psT[:, kt, :],
                                    q_t[:, kt * P:(kt + 1) * P], ident)
                nc.vector.tensor_copy(qT3[:, kt, :], psT[:, kt, :])
            return qT3

        def quant(blk):
            """LN1 + int8 fake-quant + transpose for one token block."""
            rows = slice(blk * P, (blk + 1) * P)
            x_t = xp.tile([P, DIN], f32)
            nc.scalar.dma_start(out=x_t, in_=xs[rows, :])

            ssum = stp.tile([P, 1], f32)
            nc.vector.reduce_sum(out=ssum, in_=x_t, axis=X)
            negmu = stp.tile([P, 1], f32)
            nc.vector.tensor_scalar_mul(negmu, ssum, -1.0 / DIN)

            xc_t = xcp.tile([P, DIN], f32)          # x - mu
            nc.scalar.activation(out=xc_t, in_=x_t, func=Identity,
                                 bias=negmu, scale=1.0)
            amax = stp.tile([P, 1], f32)            # max |x - mu|
            nc.vector.tensor_reduce(out=amax, in_=xc_t, axis=X,
                                    op=Alu.max, apply_absolute_value=True)

            c127 = stp.tile([P, 1], f32)            # 127 / amax
            nc.vector.reciprocal(out=c127, in_=amax)
            nc.vector.tensor_scalar_mul(c127, c127, 127.0)

            # t = xc*c + MAGIC  (rounds to integer, RNE);  q = t - MAGIC
            q_t = qp.tile([P, DIN], bf16)
            nc.scalar.activation(out=xc_t, in_=xc_t, func=Identity,
                                 bias=magic_t, scale=c127)
            nc.vector.tensor_scalar(q_t, xc_t, MAGIC, None,
                                    op0=Alu.subtract)

            # contraction-major for the PE: qT3[:, kt, :] = q[:, kt*128:+128].T
            qT3 = qtp.tile([P, KT, P], bf16)
            per = KT // 2
            for s in range(2):
                nc.sync.dma_start_transpose(
                    out=qT3[:, s * per:(s + 1) * per, :],
                    in_=q_t[:, s * per * P:(s + 1) * per * P])
            return qT3

        with tc.high_priority():
            qT_cur = quant0()
        for blk in range(NBLK):
            rows = slice(blk * P, (blk + 1) * P)
            qT_next = quant(blk + 1) if blk + 1 < NBLK else None

            ps = psp.tile([P, DOUT], f32, name="ps", tag="ps")
            st2 = stp.tile([P, NT, 6], f32)
            last = blk == NBLK - 1
            if last:
                # nt-outer: each psum bank completes early so its stats
                # overlap the remaining banks' matmuls (shorter drain).
                for nt in range(NT):
                    ncols = slice(nt * 512, (nt + 1) * 512)
                    for kt in range(KT):
                        nc.tensor.matmul(ps[:, ncols],
                                         lhsT=qT_cur[:, kt, :],
                                         rhs=w_rhs(kt, ncols),
                                         start=(kt == 0), stop=(kt == KT - 1))
                    nc.vector.bn_stats(out=st2[:, nt, :], in_=ps[:, ncols])
            else:
                # kt-outer keeps weight-chunk demand sequential so the
                # streamed weight load stays ahead of the PE on the first
                # blocks.
                for kt in range(KT):
                    for nt in range(NT):
                        ncols = slice(nt * 512, (nt + 1) * 512)
                        nc.tensor.matmul(ps[:, ncols],
                                         lhsT=qT_cur[:, kt, :],
                                         rhs=w_rhs(kt, ncols),
                                         start=(kt == 0), stop=(kt == KT - 1))
                for nt in range(NT):
                    nc.vector.bn_stats(out=st2[:, nt, :],
                                       in_=ps[:, nt * 512:(nt + 1) * 512])

            mv2 = stp.tile([P, 2], f32)
            nc.vector.bn_aggr(out=mv2, in_=st2)
            rstd2 = stp.tile([P, 1], f32)
            nc.scalar.activation(out=rstd2, in_=mv2[:, 1:2], func=Sqrt,
                                 bias=eps_t, scale=1.0)
            nc.vector.reciprocal(out=rstd2, in_=rstd2)
            nb2 = stp.tile([P, 1], f32)
            nc.vector.tensor_scalar_mul(nb2, mv2[:, 0:1], -1.0)
            nc.vector.tensor_mul(nb2, nb2, rstd2)

            o_t = op_.tile([P, DOUT], bf16)
            if last:
                # split apply+store so the final store isn't one long
                # serialized chain after the last matmul
                for h in range(2):
                    cols = slice(h * DOUT // 2, (h + 1) * DOUT // 2)
                    nc.scalar.activation(out=o_t[:, cols], in_=ps[:, cols],
                                         func=Identity, bias=nb2, scale=rstd2)
                    nc.scalar.dma_start(out=out[rows, cols], in_=o_t[:, cols])
            else:
                nc.scalar.activation(out=o_t, in_=ps, func=Identity,
                                     bias=nb2, scale=rstd2)
                nc.scalar.dma_start(out=out[rows, :], in_=o_t)

            qT_cur = qT_next

    nc.compile()
    _CACHE[key] = nc
    return nc


def _prep_in_maps(x, weight_ternary, weight_scale):
    xs = np.ascontiguousarray(
        np.asarray(x, dtype=np.float32).reshape(M_TOTAL, DIN))
    w = (np.asarray(weight_ternary).astype(np.float32)
         * np.asarray(weight_scale, dtype=np.float32)[:, None])
    wt = np.ascontiguousarray(w.T.astype(ml_dtypes.bfloat16))
    return [
        {"xs": np.ascontiguousarray(xs[c * M_PER_CORE:(c + 1) * M_PER_CORE]),
         "wt": wt}
        for c in range(N_CORES)
    ]


def run(x, weight_ternary, weight_scale, trace=False):
    from concourse.bass_utils import run_bass_kernel_spmd
    nc = _build_nc()
    in_maps = _prep_in_maps(x, weight_ternary, weight_scale)
    res = run_bass_kernel_spmd(nc, in_maps, core_ids=list(range(N_CORES)),
                               trace=trace)
    full = np.concatenate([res.results[c]["out"] for c in range(N_CORES)],
                          axis=0)
    return full.reshape(B, S, DOUT).astype(np.float32), res


def kernel(x, weight_ternary, weight_scale):
    out, _ = run(x, weight_ternary, weight_scale, trace=False)
    return out


# revision 41
# speedup vs baseline: 1.1396x; 1.1396x over previous
"""BitLinear158 (LayerNorm -> int8 fake-quant -> ternary matmul -> LayerNorm)
on 8 Trainium2 NeuronCores, data-parallel over tokens.

Math notes (vs the fp32 reference):
  - Input LayerNorm's rstd cancels inside the activation quantizer:
        q = round(xn / (max|xn|/127)) = round((x-mu) * 127 / max|x-mu|)
    so the input-side sqrt/reciprocal of the variance is never needed.
  - q in [-127,127] is exact in bf16 and the PE accumulates in fp32.
  - weight_scale is folded into the bf16 weights host-side
    (w = ternary * scale, rounded to bf16).  The final LayerNorm is
    invariant to per-token scales, and the bf16 rounding of the scaled
    ternary weights contributes ~1e-3 relative error -- well inside the
    2e-2 gate.
  - Output is stored as bf16 (LN output is O(1); bf16 adds ~2e-3 rel)
    and upcast to fp32 on the host.
  - round-half-to-even via the fp32 magic-number trick:
    t = fma(v, c, 1.5*2^23); q = t - 1.5*2^23.

Schedule notes:
  - Weights stream in 4 x 2MiB per-chunk DMAs (separate tiles, separate
    completion semaphores) on the gpsimd sw-DGE queue, so early matmuls
    only wait for the chunks they read instead of the whole 8 MiB.
    Fewer chunks also hold fewer DMA-semaphore IDs for the kernel's
    lifetime, easing the rotating semaphore pool the x/store/transpose
    DMAs recycle through (8-chunk and 2-chunk variants both measure
    slower).
  - Queue separation: x loads + output stores ride the scalar HW-DGE
    queue; the sync HW-DGE queue carries ONLY the q transposes.  With x
    loads on sync the transposes queued behind them and the first matmul
    slipped ~10us.
  - Block 0 is special-cased (quant0): half-column x loads on two queues,
    an all-vector LN/quant chain, and PE transposes (via identity matmul
    into a psum scratch slot) instead of the xbar DMA transpose, shaving
    ~10us off the pipeline ramp.  A burst of dummy matmuls warms the PE
    pstate clock during the ramp.
  - quant(b+1) is emitted before the psum-stats of block b so the vector
    engine is not head-of-line blocked on a psum semaphore; the last
    block runs nt-outer matmuls with per-bank stats and a split
    apply/store to shorten the drain.
"""

from contextlib import ExitStack

import numpy as np
import ml_dtypes

N_CORES = 8
B, S, DIN, DOUT = 4, 4096, 2048, 2048
M_TOTAL = B * S
M_PER_CORE = M_TOTAL // N_CORES
P = 128
NBLK = M_PER_CORE // P          # token blocks per core
KT = DIN // P                   # contraction subtiles
NT = DOUT // 512                # psum bank tiles
WCHUNK = 4                      # kt per weight-stream chunk
EPS = 1e-5
MAGIC = float(np.float32(1.5 * 2 ** 23))

_CACHE = {}


def _build_nc(m_per_core=M_PER_CORE):
    key = ("nc", m_per_core)
    if key in _CACHE:
        return _CACHE[key]
    NBLK = m_per_core // P

    import concourse.bacc as bacc
    import concourse.tile as tile
    from concourse import mybir

    f32 = mybir.dt.float32
    bf16 = mybir.dt.bfloat16
    X = mybir.AxisListType.X
    Identity = mybir.ActivationFunctionType.Identity
    Sqrt = mybir.ActivationFunctionType.Sqrt
    Alu = mybir.AluOpType

    nc = bacc.Bacc("TRN2", target_bir_lowering=False, num_devices=N_CORES,
                   name="bitlinear158")
    xs = nc.dram_tensor("xs", [m_per_core, DIN], f32, kind="ExternalInput")
    wt = nc.dram_tensor("wt", [DIN, DOUT], bf16, kind="ExternalInput")
    out = nc.dram_tensor("out", [m_per_core, DOUT], bf16,
                         kind="ExternalOutput")

    with tile.TileContext(nc) as tc, ExitStack() as ctx:
        singles = ctx.enter_context(tc.tile_pool(name="singles", bufs=1))
        xp = ctx.enter_context(tc.tile_pool(name="xp", bufs=2))
        xcp = ctx.enter_context(tc.tile_pool(name="xcp", bufs=3))
        qp = ctx.enter_context(tc.tile_pool(name="qp", bufs=3))
        qtp = ctx.enter_context(tc.tile_pool(name="qtp", bufs=3))
        op_ = ctx.enter_context(tc.tile_pool(name="op", bufs=3))
        stp = ctx.enter_context(tc.tile_pool(name="stp", bufs=16))
        psp = ctx.enter_context(tc.tile_pool(name="psp", bufs=2, space="PSUM"))

        # One tile per weight chunk: each gets its own completion semaphore,
        # so matmuls on early kt don't wait for the whole 8 MiB load.
        wtr = wt.rearrange("(kt p) n -> p kt n", p=P)
        w_sbs = []
        for c in range(KT // WCHUNK):
            ks = slice(c * WCHUNK, (c + 1) * WCHUNK)
            w_c = singles.tile([P, WCHUNK, DOUT], bf16,
                               name=f"w_sb{c}", tag=f"w{c}")
            nc.gpsimd.dma_start(out=w_c, in_=wtr[:, ks, :])
            w_sbs.append(w_c)

        def w_rhs(kt, ncols):
            return w_sbs[kt // WCHUNK][:, kt % WCHUNK, ncols]

        from concourse import masks
        ident = singles.tile([P, P], bf16)
        masks.make_identity(nc, ident)
        eps_t = singles.tile([P, 1], f32)
        nc.vector.memset(eps_t, EPS)
        magic_t = singles.tile([P, 1], f32)
        nc.vector.memset(magic_t, MAGIC)

        # PE warm-up: dummy matmuls against weight chunk 0 while the first
        # block's LN/quant chain runs, so the PE pstate clock is at full
        # speed when the real matmuls arrive.  Results are discarded.
        zlhs = singles.tile([P, P], bf16)
        nc.vector.memset(zlhs, 0.0)
        ps_warm = psp.tile([P, DOUT], f32, name="ps", tag="ps")
        with tc.high_priority():
            for r in range(12):
                nc.tensor.matmul(ps_warm[:, 0:512], lhsT=zlhs,
                                 rhs=w_sbs[0][:, 0, 0:512],
                                 start=True, stop=True)

        def quant0():
            """Block 0, column-split in halves: both half-loads go out on
            separate DMA queues at t=0 and every op is half-width, so the
            critical chain to the first matmul is roughly halved."""
            H = DIN // 2
            x_t = xp.tile([P, DIN], f32)
            nc.sync.dma_start(out=x_t[:, :H], in_=xs[0:P, :H])
            nc.scalar.dma_start(out=x_t[:, H:], in_=xs[0:P, H:])

            sl = stp.tile([P, 1], f32)
            nc.vector.reduce_sum(out=sl, in_=x_t[:, :H], axis=X)
            sr = stp.tile([P, 1], f32)
            nc.vector.reduce_sum(out=sr, in_=x_t[:, H:], axis=X)
            negmu = stp.tile([P, 1], f32)
            nc.vector.tensor_add(negmu, sl, sr)
            nc.vector.tensor_scalar_mul(negmu, negmu, -1.0 / DIN)

            xc_t = xcp.tile([P, DIN], f32)
            amax = stp.tile([P, 1], f32)
            am_r = stp.tile([P, 1], f32)
            nc.vector.tensor_scalar(xc_t[:, :H], x_t[:, :H], negmu, None,
                                    op0=Alu.add)
            nc.vector.tensor_reduce(out=amax, in_=xc_t[:, :H], axis=X,
                                    op=Alu.max, apply_absolute_value=True)
            nc.vector.tensor_scalar(xc_t[:, H:], x_t[:, H:], negmu, None,
                                    op0=Alu.add)
            nc.vector.tensor_reduce(out=am_r, in_=xc_t[:, H:], axis=X,
                                    op=Alu.max, apply_absolute_value=True)
            nc.vector.tensor_max(amax, amax, am_r)

            c127 = stp.tile([P, 1], f32)
            nc.vector.reciprocal(out=c127, in_=amax)
            nc.vector.tensor_scalar_mul(c127, c127, 127.0)

            q_t = qp.tile([P, DIN], bf16)
            qT3 = qtp.tile([P, KT, P], bf16)
            for s in range(2):
                cols = slice(s * H, (s + 1) * H)
                nc.vector.tensor_scalar(xc_t[:, cols], xc_t[:, cols], c127,
                                        MAGIC, op0=Alu.mult, op1=Alu.add)
                nc.vector.tensor_scalar(q_t[:, cols], xc_t[:, cols], MAGIC,
                                        None, op0=Alu.subtract)
            # PE transposes (psum scratch) instead of DMA transposes: the
            # xbar path fires ~10us after its deps are met; the PE path
            # feeds the first matmuls as each 128x128 chunk lands.
            psT = psp.tile([P, KT, P], bf16, name="ps", tag="ps")
            for kt in range(KT):
                nc.tensor.transpose(psT[:, kt, :],
                                    q_t[:, kt * P:(kt + 1) * P], ident)
                nc.vector.tensor_copy(qT3[:, kt, :], psT[:, kt, :])
            return qT3

        def quant(blk):
            """LN1 + int8 fake-quant + transpose for one token block."""
            rows = slice(blk * P, (blk + 1) * P)
            x_t = xp.tile([P, DIN], f32)
            nc.scalar.dma_start(out=x_t, in_=xs[rows, :])

            ssum = stp.tile([P, 1], f32)
            nc.vector.reduce_sum(out=ssum, in_=x_t, axis=X)
            negmu = stp.tile([P, 1], f32)
            nc.vector.tensor_scalar_mul(negmu, ssum, -1.0 / DIN)

            xc_t = xcp.tile([P, DIN], f32)          # x - mu
            nc.scalar.activation(out=xc_t, in_=x_t, func=Identity,
                                 bias=negmu, scale=1.0)
            amax = stp.tile([P, 1], f32)            # max |x - mu|
            nc.vector.tensor_reduce(out=amax, in_=xc_t, axis=X,
                                    op=Alu.max, apply_absolute_value=True)

            c127 = stp.tile([P, 1], f32)            # 127 / amax
            nc.vector.reciprocal(out=c127, in_=amax)
            nc.vector.tensor_scalar_mul(c127, c127, 127.0)

            # t = xc*c + MAGIC  (rounds to integer, RNE);  q = t - MAGIC
            q_t = qp.tile([P, DIN], bf16)
            nc.scalar.activation(out=xc_t, in_=xc_t, func=Identity,
                                 bias=magic_t, scale=c127)
            nc.vector.tensor_scalar(q_t, xc_t, MAGIC, None,
                                    op0=Alu.subtract)

            # contraction-major for the PE: qT3[:, kt, :] = q[:, kt*128:+128].T
            qT3 = qtp.tile([P, KT, P], bf16)
            per = KT // 2
            for s in range(2):
                nc.sync.dma_start_transpose(
                    out=qT3[:, s * per:(s + 1) * per, :],
                    in_=q_t[:, s * per * P:(s + 1) * per * P])
            return qT3

        with tc.high_priority():
            qT_cur = quant0()
        for blk in range(NBLK):
            rows = slice(blk * P, (blk + 1) * P)
            qT_next = quant(blk + 1) if blk + 1 < NBLK else None

            ps = psp.tile([P, DOUT], f32, name="ps", tag="ps")
            st2 = stp.tile([P, NT, 6], f32)
            last = blk == NBLK - 1
            if last:
                # nt-outer: each psum bank completes early so its stats
                # overlap the remaining banks' matmuls (shorter drain).
                for nt in range(NT):
                    ncols = slice(nt * 512, (nt + 1) * 512)
                    for kt in range(KT):
                        nc.tensor.matmul(ps[:, ncols],
                                         lhsT=qT_cur[:, kt, :],
                                         rhs=w_rhs(kt, ncols),
                                         start=(kt == 0), stop=(kt == KT - 1))
                    nc.vector.bn_stats(out=st2[:, nt, :], in_=ps[:, ncols])
            else:
                # kt-outer keeps weight-chunk demand sequential so the
                # streamed weight load stays ahead of the PE on the first
                # blocks.
                for kt in range(KT):
                    for nt in range(NT):
                        ncols = slice(nt * 512, (nt + 1) * 512)
                        nc.tensor.matmul(ps[:, ncols],
                                         lhsT=qT_cur[:, kt, :],
                                         rhs=w_rhs(kt, ncols),
                                         start=(kt == 0), stop=(kt == KT - 1))
                for nt in range(NT):
                    nc.vector.bn_stats(out=st2[:, nt, :],
                                       in_=ps[:, nt * 512:(nt + 1) * 512])

            mv2 = stp.tile([P, 2], f32)
            nc.vector.bn_aggr(out=mv2, in_=st2)
            rstd2 = stp.tile([P, 1], f32)
            nc.scalar.activation(out=rstd2, in_=mv2[:, 1:2], func=Sqrt,
                                 bias=eps_t, scale=1.0)
            nc.vector.reciprocal(out=rstd2, in_=rstd2)
            nb2 = stp.tile([P, 1], f32)
            nc.vector.tensor_scalar_mul(nb2, mv2[:, 0:1], -1.0)
            nc.vector.tensor_mul(nb2, nb2, rstd2)

            o_t = op_.tile([P, DOUT], bf16)
            if last:
                # split apply+store so the final store isn't one long
                # serialized chain after the last matmul
                for h in range(2):
                    cols = slice(h * DOUT // 2, (h + 1) * DOUT // 2)
                    nc.scalar.activation(out=o_t[:, cols], in_=ps[:, cols],
                                         func=Identity, bias=nb2, scale=rstd2)
                    nc.scalar.dma_start(out=out[rows, cols], in_=o_t[:, cols])
            else:
                nc.scalar.activation(out=o_t, in_=ps, func=Identity,
                                     bias=nb2, scale=rstd2)
                nc.scalar.dma_start(out=out[rows, :], in_=o_t)

            qT_cur = qT_next

    nc.compile()
    _CACHE[key] = nc
    return nc


def _prep_in_maps(x, weight_ternary, weight_scale):
    xs = np.ascontiguousarray(
        np.asarray(x, dtype=np.float32).reshape(M_TOTAL, DIN))
    w = (np.asarray(weight_ternary).astype(np.float32)
         * np.asarray(weight_scale, dtype=np.float32)[:, None])
    wt = np.ascontiguousarray(w.T.astype(ml_dtypes.bfloat16))
    return [
        {"xs": np.ascontiguousarray(xs[c * M_PER_CORE:(c + 1) * M_PER_CORE]),
         "wt": wt}
        for c in range(N_CORES)
    ]


def run(x, weight_ternary, weight_scale, trace=False):
    from concourse.bass_utils import run_bass_kernel_spmd
    nc = _build_nc()
    in_maps = _prep_in_maps(x, weight_ternary, weight_scale)
    res = run_bass_kernel_spmd(nc, in_maps, core_ids=list(range(N_CORES)),
                               trace=trace)
    full = np.concatenate([res.results[c]["out"] for c in range(N_CORES)],
                          axis=0)
    return full.reshape(B, S, DOUT).astype(np.float32), res


def kernel(x, weight_ternary, weight_scale):
    out, _ = run(x, weight_ternary, weight_scale, trace=False)
    return out
